# revision 8
# baseline (speedup 1.0000x reference)
"""Distributed Trainium2 kernel for a single attention head.

Problem: x:[8,2048,1024] f32, w_q/w_k/w_v:[1024,64] f32
  q,k,v = x@w ; scores = (q k^T)/sqrt(1024) causal-masked; out = softmax(scores)@v

Sharding: data-parallel over batch B=8 across the 8 NeuronCores (one batch
element per core, weights replicated, no collectives).

Per-core dataflow (T=2048, C=1024, H=64):
  - host ships x^T pre-tiled [128, chunk, c, 512] in bf16, packed w_qk / w_v
    (bf16), a 0/1 triangular mask tile, and identities for transposes.
  - ~10 warm-up matmuls on a scratch tile run while the input DMA streams so
    the PE HAM clock-gate reaches 8/8 before real work arrives.
  - all x/weight input DMAs ride the sync HWDGE queue (large transfers,
    chunk-0 split fine for early start); q/k partition-duplication and output
    DMAs ride the gpsimd SWDGE queue so they never queue behind the x stream.
  - projections with weights stationary (bf16): qkT [128,T] (q rows 0:64,
    k rows 64:128), vT [64,T]; q/k copied to SBUF and duplicated onto both
    partition halves so score pairs can run 2x row-packed on the PE.
  - scores computed TRANSPOSED per s-tile: S[s,t] = kT_slice.T @ qT (K=64),
    two s-tiles concurrently in PE row-groups 0/1; diagonal s-tiles only
    compute the columns t >= 128*rel that survive the causal mask.
  - exp on ScalarE with scale=1/32 folded in (|scores|<~2, no max needed),
    one [128,1024] activation per pair, output bf16; the ACT table set is
    pre-loaded by a dummy exp at t~0.
  - causal: diagonal 128x128 blocks multiplied by a 0/1 lower-triangle mask
    on VectorE after the exp (keeps the PE free of mask matmuls).
  - PV: out^T[h,t] accumulated over s-tiles with lhsT = [v | 1] so row 64 of
    the accumulator is the softmax denominator (fused row-sum).
  - epilogue: TensorE transpose back to [t,h], reciprocal-multiply on
    VectorE, one combined [512,64] DMA out per chunk.
  - next-chunk projections are emission-interleaved between attention pairs
    so the PE stream stays dense.
"""

import os
import sys

import numpy as np

for p in ("/opt/trn_rl_repo",):
    if p not in sys.path and os.path.isdir(p):
        sys.path.insert(0, p)

import ml_dtypes  # noqa: E402

B, T, C, H = 8, 2048, 1024, 64
N_CORES = 8
TCH = 512                  # t-chunk (columns per PSUM bank of f32)
N_CHUNK = T // TCH         # 4
N_CT = C // 128            # 8 contraction tiles
SCALE = float(C) ** -0.5   # 1/32
N_WARM = 10                # PE warm-up matmuls

_CACHE = {}


def _build():
    """Build + compile the SPMD Bass graph (same graph on all 8 cores)."""
    import concourse.bass as bass
    import concourse.mybir as mybir
    import concourse.tile as tile
    from concourse import bacc

    f32 = mybir.dt.float32
    bf16 = mybir.dt.bfloat16
    EXP = mybir.ActivationFunctionType.Exp

    nc = bacc.Bacc(
        "TRN2", target_bir_lowering=False, debug=False, num_devices=N_CORES
    )

    # host ships x^T pre-tiled: [128, N_CHUNK * N_CT * TCH] laid out
    # [chunk][c-tile][t] per partition so each chunk is one contiguous DMA.
    xT_d = nc.dram_tensor("xT", [128, N_CHUNK * N_CT * TCH], bf16, kind="ExternalInput")
    wqk_d = nc.dram_tensor("wqk", [128, N_CT * 128], bf16, kind="ExternalInput")
    wv_d = nc.dram_tensor("wv", [128, N_CT * H], bf16, kind="ExternalInput")
    mask_d = nc.dram_tensor("mask01", [128, 128], bf16, kind="ExternalInput")  # 0/1, keep s<=t
    idf_d = nc.dram_tensor("idf", [128, 128], f32, kind="ExternalInput")
    idb_d = nc.dram_tensor("idb", [128, 128], bf16, kind="ExternalInput")
    out_d = nc.dram_tensor("out", [T, H], f32, kind="ExternalOutput")

    with tile.TileContext(nc) as tc:
        with (
            tc.tile_pool(name="const", bufs=1) as constp,
            tc.tile_pool(name="xTp", bufs=1) as xTp,
            tc.tile_pool(name="qkp", bufs=1) as qkp,
            tc.tile_pool(name="q2p", bufs=2) as q2p,
            tc.tile_pool(name="vTp", bufs=2) as vTp,
            tc.tile_pool(name="v1p", bufs=1) as v1p,
            tc.tile_pool(name="exp", bufs=4) as expp,
            tc.tile_pool(name="epi", bufs=2) as epip,
            tc.tile_pool(name="Sp", bufs=2, space="PSUM") as Sp,
            tc.tile_pool(name="accp", bufs=1, space="PSUM") as accp,
            tc.tile_pool(name="miscp", bufs=3, space="PSUM") as miscp,
        ):
            # ---- PE warm-up + ACT table pre-load scratch ----
            warm_sb = constp.tile([128, 256], bf16, tag="warm_sb", name="warm_sb")
            nc.vector.memset(warm_sb[:], 0.0)
            warm_act = constp.tile([128, 8], bf16, tag="warm_act", name="warm_act")
            warm_ps = miscp.tile([128, 256], f32, tag="misc", name="warm_ps")
            for i in range(N_WARM):
                nc.tensor.matmul(
                    warm_ps[:, :],
                    warm_sb[:, 0:128],
                    warm_sb[:, :],
                    start=True,
                    stop=True,
                    skip_group_check=True,
                )
            # dummy exp: forces the ACT table set load during the DMA phase
            nc.scalar.activation(warm_act[:], warm_sb[:, 0:8], EXP, scale=1.0)

            # ---- input DMAs: weights + x chunks, split across the two HWDGE
            # queues (sync + scalar) so issue overhead parallelizes. Chunk 0
            # is split in 4 pieces so the first projection matmuls start as
            # soon as the first c-tiles land. ----
            wqk_t = constp.tile([128, N_CT, 128], bf16, tag="wqk", name="wqk_t")
            nc.sync.dma_start(
                out=wqk_t[:], in_=wqk_d[:].rearrange("p (n m) -> p n m", n=N_CT)
            )
            xt = {}
            for t in range(N_CHUNK):
                xt[t] = xTp.tile([128, N_CT, TCH], bf16, tag=f"x{t}", name=f"x{t}")
            xT_v = xT_d[:].rearrange("p (t n m) -> p t n m", t=N_CHUNK, n=N_CT)
            for h in range(4):  # chunk 0 in 4 pieces of 2 c-tiles
                eng = nc.sync if h < 2 else nc.scalar
                eng.dma_start(
                    out=xt[0][:, 2 * h : 2 * h + 2, :], in_=xT_v[:, 0, 2 * h : 2 * h + 2, :]
                )
            wv_t = constp.tile([128, N_CT, H], bf16, tag="wv", name="wv_t")
            nc.scalar.dma_start(
                out=wv_t[:], in_=wv_d[:].rearrange("p (n m) -> p n m", n=N_CT)
            )
            for t, eng in ((1, nc.scalar), (2, nc.sync), (3, nc.scalar)):
                eng.dma_start(out=xt[t][:], in_=xT_v[:, t, :, :])

            # ---- small constants on the gpsimd SWDGE queue ----
            mask_t = constp.tile([128, 128], bf16, tag="mask", name="mask_t")
            nc.gpsimd.dma_start(out=mask_t[:], in_=mask_d[:])
            idb_t = constp.tile([128, 128], bf16, tag="idb", name="idb_t")
            nc.gpsimd.dma_start(out=idb_t[:], in_=idb_d[:])
            idf_t = constp.tile([128, 128], f32, tag="idf", name="idf_t")
            nc.gpsimd.dma_start(out=idf_t[:], in_=idf_d[:])

            qk2 = {}   # [128, TCH] bf16: qT duplicated on both partition halves
            kk2 = {}   # [128, TCH] bf16: kT duplicated on both partition halves
            v1 = {}    # [128, 65] bf16 per s-tile: [v | 1]

            def proj_steps(tch):
                """Emission thunks for chunk `tch`: (qk steps, v steps)."""
                qk_steps = []
                v_steps = []
                state = {}

                def qk_mm(c):
                    def f():
                        if c == 0:
                            state["S"] = miscp.tile(
                                [128, TCH], f32, tag="misc", name=f"Sqk{tch}"
                            )
                        nc.tensor.matmul(
                            state["S"][:, :],
                            wqk_t[:, c, :],
                            xt[tch][:, c, :],
                            start=(c == 0),
                            stop=(c == N_CT - 1),
                            skip_group_check=True,
                        )
                    return f

                def qk_out():
                    S = state["S"]
                    q2 = q2p.tile([128, TCH], bf16, tag="q2", name=f"q2_{tch}")
                    k2 = qkp.tile([128, TCH], bf16, tag=f"k2_{tch}", name=f"k2_{tch}")
                    nc.vector.tensor_copy(k2[0:64, :], S[64:128, :])
                    nc.vector.tensor_copy(q2[0:64, :], S[0:64, :])
                    # chunk 0's own scores run unpacked on partition half 0,
                    # so its q never needs duplicating; k still does (used by
                    # later chunks in row-packed pairs).
                    if tch > 0:
                        nc.gpsimd.dma_start(out=q2[64:128, :], in_=q2[0:64, :])
                    nc.gpsimd.dma_start(out=k2[64:128, :], in_=k2[0:64, :])
                    qk2[tch] = q2
                    kk2[tch] = k2

                def v_mm(c):
                    def f():
                        if c == 0:
                            state["Pv"] = miscp.tile(
                                [64, TCH], f32, tag="misc", name=f"Pv{tch}"
                            )
                        nc.tensor.matmul(
                            state["Pv"][:, :],
                            wv_t[:, c, :],
                            xt[tch][:, c, :],
                            start=(c == 0),
                            stop=(c == N_CT - 1),
                            skip_group_check=True,
                        )
                    return f

                def v_out():
                    vTt = vTp.tile([64, TCH], bf16, tag="vT", name=f"vT{tch}")
                    nc.vector.tensor_copy(vTt[:], state["Pv"][:, :])
                    state["vT"] = vTt

                def v1_build(i):
                    def f():
                        j = 4 * tch + i
                        Pt = miscp.tile([128, H], bf16, tag="misc", name=f"Pt{j}")
                        nc.tensor.transpose(
                            Pt[:, :],
                            state["vT"][:, 128 * i : 128 * (i + 1)],
                            idb_t[0:64, 0:64],
                        )
                        v1t = v1p.tile([128, 65], bf16, tag=f"v1_{j}", name=f"v1_{j}")
                        nc.vector.tensor_copy(v1t[:, 0:64], Pt[:, :])
                        nc.vector.memset(v1t[:, 64:65], 1.0)
                        v1[j] = v1t
                    return f

                for c in range(N_CT):
                    qk_steps.append(qk_mm(c))
                qk_steps.append(qk_out)
                for c in range(N_CT):
                    v_steps.append(v_mm(c))
                v_steps.append(v_out)
                for i in range(4):
                    v_steps.append(v1_build(i))
                return qk_steps, v_steps

            def emit_scores_exp(tch, jp, unpacked):
                """Scores matmuls + exp for pair (jp, jp+1); returns (ext, los)."""
                S2 = Sp.tile([128, 2 * TCH], f32, tag="S", name=f"S{tch}_{jp}")
                los = {}
                for jj in range(2):
                    j = jp + jj
                    rel = j - 4 * tch
                    lo = 128 * max(0, rel)
                    los[jj] = lo
                    half = slice(0, 64) if unpacked else slice(64 * jj, 64 * (jj + 1))
                    ksl = kk2[j // 4][half, 128 * (j % 4) : 128 * (j % 4 + 1)]
                    nc.tensor.matmul(
                        S2[:, TCH * jj + lo : TCH * (jj + 1)],
                        ksl,
                        qk2[tch][half, lo:TCH],
                        start=True,
                        stop=True,
                        skip_group_check=True,
                    )
                ext = expp.tile([128, 2 * TCH], bf16, tag="ex", name=f"ex{tch}_{jp}")
                nc.scalar.activation(ext[:], S2[:], EXP, scale=SCALE)
                # causal 0/1 mask on the diagonal 128x128 blocks (VectorE)
                for jj in range(2):
                    j = jp + jj
                    if j - 4 * tch >= 0:
                        a = TCH * jj + los[jj]
                        nc.vector.tensor_mul(
                            ext[:, a : a + 128], ext[:, a : a + 128], mask_t[:]
                        )
                return ext, los

            def emit_pv(tch, jp, acc, ext, los):
                jmax = 4 * tch + 3
                for jj in range(2):
                    j = jp + jj
                    lo = los[jj]
                    nc.tensor.matmul(
                        acc[:, lo:TCH] if j > 0 else acc[:, :],
                        v1[j][:],
                        ext[:, TCH * jj + lo : TCH * (jj + 1)],
                        start=(j == 0),
                        stop=(j == jmax),
                        skip_group_check=True,
                    )

            # ---- chunk 0: qk proj, then scores+exp of pair 0 immediately
            # (unpacked, no q/k duplication round-trip on the critical path),
            # with the v projection filling the PE while the exp runs. ----
            qk0, v0 = proj_steps(0)
            for s in qk0:
                s()
            acc = accp.tile([65, TCH], f32, tag="acc", name="acc0")
            ext0, los0 = emit_scores_exp(0, 0, unpacked=True)
            for s in v0:
                s()
            ext1, los1 = emit_scores_exp(0, 2, unpacked=True)
            emit_pv(0, 0, acc, ext0, los0)
            qkn, vn = proj_steps(1)
            # chunk-1 qk projection (+ q/k copies) rides between chunk-0 PVs
            pending = vn
            for s in qkn:
                s()
            emit_pv(0, 2, acc, ext1, los1)

            for tch in range(N_CHUNK):
                if tch > 0:
                    # chunk tch's remaining projection work must be emitted
                    # before its own pairs reference q2/k2/v1 tiles
                    for s in pending:
                        s()
                    pending = []
                    if tch + 1 < N_CHUNK:
                        qkn, vn = proj_steps(tch + 1)
                        pending = qkn + vn
                    jmax = 4 * tch + 3
                    pairs = list(range(0, jmax + 1, 2))
                    per_pair = -(-len(pending) // len(pairs)) if pending else 0
                    acc = accp.tile([65, TCH], f32, tag="acc", name=f"acc{tch}")
                    for jp in pairs:
                        ext, los = emit_scores_exp(tch, jp, unpacked=False)
                        emit_pv(tch, jp, acc, ext, los)
                        # emit a slice of next-chunk projection work
                        for _ in range(per_pair):
                            if pending:
                                pending.pop(0)()
                    for s in pending:
                        s()
                    pending = []

                # ======== epilogue: normalize + transpose + DMA out ========
                oT = epip.tile([65, TCH], f32, tag="oT", name=f"oT{tch}")
                nc.vector.tensor_copy(oT[:], acc[:])
                ot = epip.tile([128, 4, H], f32, tag="ot", name=f"ot{tch}")
                for i in range(4):
                    Pe = miscp.tile([128, 65], f32, tag="misc", name=f"Pe{tch}_{i}")
                    nc.tensor.transpose(
                        Pe[:, :],
                        oT[:, 128 * i : 128 * (i + 1)],
                        idf_t[0:65, 0:65],
                    )
                    rec = epip.tile([128, 1], f32, tag="rec", name=f"rec{tch}_{i}")
                    nc.vector.reciprocal(rec[:], Pe[:, 64:65])
                    nc.vector.tensor_scalar_mul(ot[:, i, :], Pe[:, 0:64], rec[:])
                r0 = TCH * tch
                nc.sync.dma_start(
                    out=out_d[r0 : r0 + TCH, :].rearrange("(i p) h -> p i h", i=4),
                    in_=ot[:],
                )

    nc.compile()
    return nc


def _get_nc():
    if "nc" not in _CACHE:
        _CACHE["nc"] = _build()
    return _CACHE["nc"]


def _tile_w(w):
    """[C, F] -> [128, N_CT*F] with c-tile-major column blocks."""
    Cdim, F = w.shape
    return np.ascontiguousarray(
        w.reshape(Cdim // 128, 128, F).transpose(1, 0, 2).reshape(128, -1)
    )


def _host_inputs(x, w_q, w_k, w_v):
    bf = ml_dtypes.bfloat16
    x = np.asarray(x, dtype=np.float32)
    wqk = np.concatenate(
        [np.asarray(w_q, np.float32), np.asarray(w_k, np.float32)], 1
    )
    wv = np.asarray(w_v, np.float32)
    wqk_tiled = _tile_w(wqk).astype(bf)
    wv_tiled = _tile_w(wv).astype(bf)
    # multiplicative causal mask for transposed-score diag blocks: keep s <= t
    mask01 = np.triu(np.ones((128, 128), np.float32)).astype(bf)
    idf = np.eye(128, dtype=np.float32)
    idb = np.eye(128, dtype=np.float32).astype(bf)
    in_maps = []
    for i in range(N_CORES):
        # x^T pre-tiled: [128, chunk, c-tile, t] flattened per partition
        xT = np.ascontiguousarray(x[i].T).astype(bf)  # [C, T]
        xT4 = xT.reshape(N_CT, 128, N_CHUNK, TCH)     # [c, p, chunk, t]
        xTt = np.ascontiguousarray(
            xT4.transpose(1, 2, 0, 3).reshape(128, -1)
        )
        in_maps.append(
            {
                "xT": xTt,
                "wqk": wqk_tiled,
                "wv": wv_tiled,
                "mask01": mask01,
                "idf": idf,
                "idb": idb,
            }
        )
    return in_maps


def run(x, w_q, w_k, w_v, trace=False, **trace_kwargs):
    from concourse.bass_utils import run_bass_kernel_spmd

    nc = _get_nc()
    in_maps = _host_inputs(x, w_q, w_k, w_v)
    res = run_bass_kernel_spmd(
        nc, in_maps, core_ids=list(range(N_CORES)), trace=trace, **trace_kwargs
    )
    out = np.stack([np.asarray(res.results[i]["out"]) for i in range(N_CORES)])
    return out.astype(np.float32), res


def kernel(x, w_q, w_k, w_v):
    out, _ = run(x, w_q, w_k, w_v, trace=False)
    return out
